# revision 11
# baseline (speedup 1.0000x reference)
"""Trainium2 Bass kernel for the nn_Experts MoE-LoRA problem.

Computes, for x = hidden_states.reshape(T, D):
    probs   = softmax(x @ Wr + br)
    w, idx  = top2(probs); combine[t,e] = w if e selected else 0
    base    = x @ W1                     (b1 folded into the gelu bias)
    t1      = einsum('td,erd->ter', x, A1)
    l1      = einsum('ter,efr->tef', t1, B1) * 2.0
    a       = gelu_tanh(base[:,None,:] + b1 + l1)
    ca      = a * combine[:,:,None]
    mix     = ca.sum(1)
    t2      = einsum('tef,erf->ter', ca, A2)
    l2      = einsum('ter,edr->td', t2, B2) * 2.0
    out     = mix @ W2 + combine.sum(-1,keepdims) * b2 + l2

Sharding: the F=8192 ff dimension is split across the 8 cores (Fs=1024
per core).  Each core holds the full token set and all 8 experts'
LoRA factors restricted to its F-slice, and produces a partial
out^T = W2s^T @ mix_s^T + l2_partial, which the host sums over cores.

v2 changes vs the 295us baseline:
  - router logits via a 3-term bf16 split (hi*hi + lo*hi + hi*lo)
    instead of fp32 LOW_HIGH matmuls: 0 top-2 flips on this data with
    a ~49x logit-gap margin, ~4x cheaper on PE, and the 8.4MB fp32
    x load disappears (xlo reuses the same byte budget as xhi).
    The 4 (k,term) sub-chains are col-packed on the PE array into 4
    independent psum accumulators, reduced with one DVE stt (+br).
  - base GEMM shares each W1 weight tile across both 512-token chunks
    (LDW + 2 matmuls), the pattern the W2 phase already runs at
    ~225ns/MM vs the unshared 385ns/MM.
  - t2 is computed from a (not ca) and scaled by combine once at the
    end in rank space, so the whole PE pipeline is router-independent;
    only the DVE-side ca/mix ops wait on the combine broadcast.
  - l1 strips are 2-expert row-packed and share strip weights across
    both token chunks; t1 shares A1 loads the same way.
  - PSUM: base pair 2 banks + 4 strip banks + 2 transient t2 = 8.
"""

import os
import sys

for _p in ("/opt/trn_rl_repo", os.path.join(os.path.dirname(os.path.abspath(__file__)))):
    if _p not in sys.path:
        sys.path.insert(0, _p)

import numpy as np
import ml_dtypes

import concourse.bass as bass
import concourse.mybir as mybir
import concourse.tile as tile
from concourse import bacc

BF16 = mybir.dt.bfloat16
F32 = mybir.dt.float32
AF = mybir.ActivationFunctionType
ALU = mybir.AluOpType
AX = mybir.AxisListType

E = 8      # experts
K = 2      # top-k
D = 2048   # hidden
F = 8192   # ff dim (full)
R = 16     # lora rank
RP = 32    # padded rank (32-aligned for PE row/col strips)
SCALING = 2.0
NCORES = 8
FS = F // NCORES   # per-core ff slice = 1024
P = 128
TCH = 512          # token chunk (one PSUM bank of fp32)


# --------------------------------------------------------------------------
# device program
# --------------------------------------------------------------------------

def build_nc(T: int) -> bass.Bass:
    assert T % TCH == 0
    n_tch = T // TCH          # 2
    n_mt = T // P             # token tiles (8)
    KT = D // P               # contraction tiles over D = 16

    nc = bacc.Bacc("TRN2", target_bir_lowering=False, debug=False,
                   num_devices=NCORES)

    # ---- DRAM parameters (per-core data) ----
    cstage = nc.dram_tensor("cstage", [9, T], BF16).ap()
    xTb = nc.dram_tensor("xTb", [D, T], BF16, kind="ExternalInput").ap()
    xTl = nc.dram_tensor("xTl", [D, T], BF16, kind="ExternalInput").ap()
    w1s = nc.dram_tensor("w1s", [(FS // P) * KT * P, P], BF16,
                         kind="ExternalInput").ap()
    w2s = nc.dram_tensor("w2s", [FS, D], BF16, kind="ExternalInput").ap()
    a1T = nc.dram_tensor("a1T", [D, P], BF16, kind="ExternalInput").ap()
    b1rT = nc.dram_tensor("b1rT", [2 * P, FS], BF16, kind="ExternalInput").ap()
    a2sT = nc.dram_tensor("a2sT", [FS, 2 * P], BF16, kind="ExternalInput").ap()
    b2rT = nc.dram_tensor("b2rT", [2 * P, D], BF16, kind="ExternalInput").ap()
    wrh = nc.dram_tensor("wrh", [D, E], BF16, kind="ExternalInput").ap()
    wrl = nc.dram_tensor("wrl", [D, E], BF16, kind="ExternalInput").ap()
    brv = nc.dram_tensor("brv", [E, 1], F32, kind="ExternalInput").ap()
    b1sM = nc.dram_tensor("b1sM", [P, FS // P], F32, kind="ExternalInput").ap()
    idf = nc.dram_tensor("idf", [P, P], F32, kind="ExternalInput").ap()
    idb = nc.dram_tensor("idb", [P, P], BF16, kind="ExternalInput").ap()
    outT = nc.dram_tensor("outT", [D, T], F32, kind="ExternalOutput").ap()

    with tile.TileContext(nc) as tc:
        _emit(tc, T, n_tch, n_mt, KT,
              xTb, xTl, w1s, w2s, a1T, b1rT, a2sT, b2rT,
              wrh, wrl, brv, b1sM, outT, cstage, idf, idb)
    nc.compile()
    return nc


def _emit(tc, T, n_tch, n_mt, KT,
          xTb, xTl, w1s, w2s, a1T, b1rT, a2sT, b2rT,
          wrh, wrl, brv, b1sM, outT, cstage, idf, idb):
    nc = tc.nc
    from contextlib import ExitStack
    ctx = ExitStack()

    # ---------------- resident pool; small consts first ----------------
    resid = ctx.enter_context(tc.tile_pool(name="resid", bufs=1))

    # router weights: whi and wlo, per k-tile [128, 8]
    wrh_t = resid.tile([P, KT * E], BF16, name="wrh_t", tag="wrh_t")
    nc.sync.dma_start(wrh_t[:].rearrange("p (k e) -> p k e", e=E),
                      wrh.rearrange("(k p) e -> p k e", p=P))
    wrl_t = resid.tile([P, KT * E], BF16, name="wrl_t", tag="wrl_t")
    nc.sync.dma_start(wrl_t[:].rearrange("p (k e) -> p k e", e=E),
                      wrl.rearrange("(k p) e -> p k e", p=P))
    wh_k = [wrh_t[:, k * E:(k + 1) * E] for k in range(KT)]
    wl_k = [wrl_t[:, k * E:(k + 1) * E] for k in range(KT)]

    brv_t = resid.tile([E, 1], F32, name="brv_t", tag="brv_t")
    nc.sync.dma_start(brv_t[:], brv[:, :])
    b1s_t = resid.tile([P, FS // P], F32, name="b1s_t", tag="b1s_t")
    nc.sync.dma_start(b1s_t[:], b1sM[:, :])
    ident = resid.tile([P, P], F32, name="ident", tag="ident")
    nc.sync.dma_start(ident[:], idf[:, :])
    ident_bf = resid.tile([P, P], BF16, name="ident_bf", tag="ident_bf")
    nc.sync.dma_start(ident_bf[:], idb[:, :])

    # x hi (bf16) resident, chunk-major loads: chunk c covers tokens
    # [c*TCH,(c+1)*TCH) for all 16 k-tiles, so router/t1/base for chunk 0
    # can start while chunk 1 is still in flight.
    xbf_all = resid.tile([P, KT * T], BF16, name="xbf_all", tag="xbf_all")
    xbf_t = [xbf_all[:, k * T:(k + 1) * T] for k in range(KT)]

    def load_x_chunk(c):
        ts = slice(c * TCH, (c + 1) * TCH)
        nc.sync.dma_start(
            xbf_all[:].rearrange("p (k t) -> p k t", t=T)[:, :, ts],
            xTb[:, ts].rearrange("(k p) t -> p k t", p=P))

    load_x_chunk(0)
    # lora factor loads follow chunk 0 of x so t1/strips can start early
    a1_all = resid.tile([P, KT * P], BF16, name="a1_all", tag="a1_all")
    nc.sync.dma_start(a1_all[:].rearrange("p (k r) -> p k r", r=P),
                      a1T.rearrange("(k p) r -> p k r", p=P))
    a1_t = [a1_all[:, k * P:(k + 1) * P] for k in range(KT)]
    load_x_chunk(1)

    b1r_t = []
    for g in range(2):
        t = resid.tile([P, FS], BF16, name=f"b1r{g}", tag=f"b1r{g}")
        nc.sync.dma_start(t[:], b1rT[g * P:(g + 1) * P, :])
        b1r_t.append(t)
    a2_all = resid.tile([P, (FS // P) * 2 * P], BF16, name="a2_all",
                        tag="a2_all")
    nc.sync.dma_start(a2_all[:].rearrange("p (f r) -> p f r", r=2 * P),
                      a2sT.rearrange("(f p) r -> p f r", p=P))
    a2_t = [a2_all[:, f * 2 * P:(f + 1) * 2 * P] for f in range(FS // P)]

    # x lo (router only) on the scalar queue so it races ahead of the
    # W1 stream rather than queueing behind x hi.  Router-phase pools
    # live in their own stack so their SBUF is reused by the main loop.
    rtr = ExitStack()
    xlo_p = rtr.enter_context(tc.tile_pool(name="xlo_p", bufs=2))
    xlo_c = []
    for c in range(n_tch):
        t = xlo_p.tile([P, KT * TCH], BF16, name="xlo", tag=f"xlo{c}", bufs=1)
        ts = slice(c * TCH, (c + 1) * TCH)
        nc.scalar.dma_start(
            t[:].rearrange("p (k t) -> p k t", t=TCH),
            xTl[:, ts].rearrange("(k p) t -> p k t", p=P))
        xlo_c.append(t)

    # combine tiles (filled by router): combine^T+csum [9, T] bf16 & fp32,
    # per-expert broadcast rows [128, T], rank-broadcast rows for t2 scale.
    cbf = resid.tile([9, T], BF16, name="cbf", tag="cbf")
    cbc_t = []
    for e in range(E):
        cbc_t.append(resid.tile([P, T], BF16, name=f"cbc{e}", tag=f"cbc{e}"))
    cbcr_t = []
    for g in range(2):
        cbcr_t.append(resid.tile([P, T], BF16, name=f"cbcr{g}", tag=f"cbcr{g}"))

    t1_t = []
    for g in range(2):
        t1_t.append(resid.tile([P, T], BF16, name=f"t1_{g}", tag=f"t1_{g}"))
    mix_all = [None] * ((FS // P) * n_tch)
    t2acc = []
    for g in range(2):
        t2acc.append(resid.tile([P, T], BF16, name=f"t2a{g}", tag=f"t2a{g}"))
    t2s = []
    for g in range(2):
        t2s.append(resid.tile([P, T], BF16, name=f"t2s{g}", tag=f"t2s{g}"))

    # ---------------- PE warmup (HAM) ----------------
    # ~25 junk transposes as soon as the identity lands so the PE clock
    # gate is already at 8/8 when the router chain arrives.
    with tc.tile_pool(name="warm_ps", bufs=1, space="PSUM") as wps:
        wt = wps.tile([P, P], F32, name="warm", tag="warm")
        for _ in range(25):
            nc.tensor.transpose(wt[:], ident[:], ident[:])

    # ---------------- router ----------------
    # logits^T [E, T] = Whi^T xhi + Whi^T xlo + Wlo^T xhi  (3-term bf16
    # split; max logit err ~1.6e-5 vs min top2/3 gap 8e-4 on this data).
    # The 48 (term, k) matmuls per chunk are col-packed 4-wide: sub-chain
    # j accumulates into psum partitions [32j, 32j+8); a DVE
    # scalar_tensor_tensor pair reduces the 4 accumulators and adds br.
    rsb = rtr.enter_context(tc.tile_pool(name="router_sb", bufs=3))
    lgT = rsb.tile([E, T], F32, name="lgT", tag="lgT", bufs=1)

    def router_chain(c):
        ts = slice(c * TCH, (c + 1) * TCH)
        with tc.tile_pool(name=f"rps{c}", bufs=1, space="PSUM") as rps:
            plg = rps.tile([P, TCH], F32, name="plg", tag="plg")
            terms = ([(wh_k[k], xbf_t[k][:, ts]) for k in range(KT)] +
                     [(wh_k[k], xlo_c[c][:, k * TCH:(k + 1) * TCH])
                      for k in range(KT)] +
                     [(wl_k[k], xbf_t[k][:, ts]) for k in range(KT)])
            nsub = (len(terms) + 3) // 4
            for j in range(4):
                sub = terms[j * nsub:(j + 1) * nsub]
                for i, (w, xx) in enumerate(sub):
                    nc.tensor.matmul(plg[RP * j:RP * j + E, :], w, xx,
                                     start=(i == 0), stop=(i == len(sub) - 1),
                                     tile_position=(0, RP * j),
                                     skip_group_check=True)
            # DVE may read at most one PSUM operand per op: copy two of
            # the four col-packed sub-accumulators through ACT first.
            cA = rsb.tile([E, TCH], F32, name="rcA", tag="rcA")
            nc.scalar.copy(cA[:], plg[0:E, :])
            cB = rsb.tile([E, TCH], F32, name="rcB", tag="rcB")
            nc.scalar.copy(cB[:], plg[2 * RP:2 * RP + E, :])
            s1 = rsb.tile([E, TCH], F32, name="rs1", tag="rs1")
            nc.vector.tensor_tensor(s1[:], cA[:], plg[RP:RP + E, :],
                                    op=ALU.add)
            s2 = rsb.tile([E, TCH], F32, name="rs2", tag="rs2")
            nc.vector.tensor_tensor(s2[:], cB[:], plg[3 * RP:3 * RP + E, :],
                                    op=ALU.add)
            # lgT = (s1 + br) + s2
            nc.vector.scalar_tensor_tensor(lgT[:, ts], s1[:], brv_t[:, 0:1],
                                           s2[:], op0=ALU.add, op1=ALU.add)

    def router_softmax(c):
        with tc.tile_pool(name=f"tp{c}", bufs=2, space="PSUM") as tps:
            for m in range(c * (n_mt // n_tch), (c + 1) * (n_mt // n_tch)):
                pr = tps.tile([P, E], F32, name="pr", tag="pr")
                nc.tensor.transpose(pr[:], lgT[:, m * P:(m + 1) * P],
                                    ident[:E, :E])
                negmax = rsb.tile([P, 1], F32, name="negmax", tag="negmax")
                nc.vector.tensor_reduce(negmax[:], pr[:], axis=AX.X,
                                        op=ALU.max, negate=True)
                pexp = rsb.tile([P, E], F32, name="pexp", tag="pexp")
                nc.scalar.activation(pexp[:], pr[:], AF.Exp,
                                     bias=negmax[:, 0:1], scale=1.0)
                ssum = rsb.tile([P, 1], F32, name="ssum", tag="ssum")
                nc.vector.tensor_reduce(ssum[:], pexp[:], axis=AX.X,
                                        op=ALU.add)
                rsum = rsb.tile([P, 1], F32, name="rsum", tag="rsum")
                nc.vector.reciprocal(rsum[:], ssum[:])
                probs = rsb.tile([P, E], F32, name="probs", tag="probs")
                nc.vector.tensor_scalar_mul(probs[:], pexp[:], rsum[:, 0:1])
                m1 = rsb.tile([P, 1], F32, name="m1", tag="m1")
                nc.vector.tensor_reduce(m1[:], probs[:], axis=AX.X, op=ALU.max)
                mask1 = rsb.tile([P, E], F32, name="mask1", tag="mask1")
                nc.vector.tensor_scalar(mask1[:], probs[:], m1[:, 0:1], None,
                                        op0=ALU.is_ge)
                pm = rsb.tile([P, E], F32, name="pm", tag="pm")
                nc.vector.scalar_tensor_tensor(pm[:], mask1[:], -2.0, probs[:],
                                               op0=ALU.mult, op1=ALU.add)
                m2 = rsb.tile([P, 1], F32, name="m2", tag="m2")
                nc.vector.tensor_reduce(m2[:], pm[:], axis=AX.X, op=ALU.max)
                mask2 = rsb.tile([P, E], F32, name="mask2", tag="mask2")
                nc.vector.tensor_scalar(mask2[:], probs[:], m2[:, 0:1], None,
                                        op0=ALU.is_ge)
                comb = rsb.tile([P, E + 1], F32, name="comb", tag="comb")
                nc.vector.tensor_tensor(comb[:, 0:E], probs[:], mask2[:],
                                        op=ALU.mult)
                nc.vector.tensor_reduce(comb[:, E:E + 1], comb[:, 0:E],
                                        axis=AX.X, op=ALU.add)
                ptp = tps.tile([E + 1, P], F32, name="ptp", tag="ptp")
                nc.tensor.transpose(ptp[:], comb[:, 0:E + 1], ident[:])
                ms = slice(m * P, (m + 1) * P)
                nc.scalar.copy(cbf[:, ms], ptp[:])

    def router_bcast(c):
        # stage through DRAM (SBUF-source partition-broadcast DMA is
        # rejected; DRAM APs are linear), per chunk so chunk-0 combine
        # rows land before the first ca multiply needs them.
        ts = slice(c * TCH, (c + 1) * TCH)
        nc.gpsimd.dma_start(cstage[:, ts], cbf[:, ts])
        for e in range(E):
            nc.gpsimd.dma_start(
                cbc_t[e][:, ts],
                cstage[e:e + 1, ts].to_broadcast([P, TCH]))
        for e in range(E):
            g, el = divmod(e, 4)
            nc.gpsimd.dma_start(
                cbcr_t[g][RP * el:RP * el + R, ts],
                cstage[e:e + 1, ts].to_broadcast([R, TCH]))

    # ---------------- t1 ----------------
    # t1un rows are the 8*16 real ranks; a strided SBUF-to-SBUF DMA
    # spreads them into the 32-aligned padded layout the strips need.
    t1un = resid.tile([P, T], BF16, name="t1un", tag="t1un")

    def t1_chunk(c):
        ts = slice(c * TCH, (c + 1) * TCH)
        with tc.tile_pool(name=f"t1ps{c}", bufs=1, space="PSUM") as t1ps:
            pt1 = t1ps.tile([P, TCH], F32, name="pt1", tag="pt1")
            for k in range(KT):
                nc.tensor.matmul(pt1[:], a1_t[k][:], xbf_t[k][:, ts],
                                 start=(k == 0), stop=(k == KT - 1))
            nc.scalar.copy(t1un[:, ts], pt1[:])

    def t1_spread():
        nc.vector.memset(t1_t[0][:], 0.0)
        nc.vector.memset(t1_t[1][:], 0.0)
        for e in range(E):
            g, el = divmod(e, 4)
            nc.sync.dma_start(t1_t[g][RP * el:RP * el + R, :],
                              t1un[R * e:R * e + R, :])

    # emission order: router c0 -> t1 c0 -> router c1 -> t1 c1 so the PE
    # never sits behind a DMA it doesn't need yet.
    router_chain(0)
    router_softmax(0)
    router_bcast(0)
    t1_chunk(0)
    router_chain(1)
    router_softmax(1)
    router_bcast(1)
    t1_chunk(1)
    t1_spread()
    rtr.close()

    # ---------------- main pipeline ----------------
    n_fs = FS // P     # 8 f-tiles per core
    n_dm = D // P      # 16 output d-tiles

    main = ctx.enter_context(tc.tile_pool(name="main_sb", bufs=3))
    mixp = ctx.enter_context(tc.tile_pool(name="mix_sb", bufs=2))
    w2p = ctx.enter_context(tc.tile_pool(name="w2_sb", bufs=4))
    outp = ctx.enter_context(tc.tile_pool(name="out_sb", bufs=2))

    w2_t = []
    b2r_t = []

    def load_phase_b_weights():
        for f in range(n_fs):
            t = w2p.tile([P, D], BF16, name=f"w2_{f}", tag=f"w2_{f}", bufs=1)
            nc.scalar.dma_start(t[:], w2s[f * P:(f + 1) * P, :])
            w2_t.append(t)
        for g in range(2):
            t = resid.tile([P, D], BF16, name=f"b2r{g}", tag=f"b2r{g}")
            nc.scalar.dma_start(t[:], b2rT[g * P:(g + 1) * P, :])
            b2r_t.append(t)

    mainps = ExitStack()
    pbp = mainps.enter_context(tc.tile_pool(name="base_ps", bufs=1,
                                            space="PSUM"))
    plp = mainps.enter_context(tc.tile_pool(name="l1_ps", bufs=4,
                                            space="PSUM"))
    pt2p = mainps.enter_context(tc.tile_pool(name="t2_ps", bufs=1,
                                             space="PSUM"))

    # t2 strip matmuls are emitted one f late so the PE never waits on
    # the ACT gelu that produces a.
    pending_t2 = []

    def flush_t2():
        for (f0, a0) in pending_t2:
            for tch in range(n_tch):
                pt2 = [None, None]
                for g in range(2):
                    pt2[g] = pt2p.tile([P, TCH], F32, name="pt2",
                                       tag=f"pt2_{g}")
                    for el in range(4):
                        e = 4 * g + el
                        nc.tensor.matmul(
                            pt2[g][RP * el:RP * el + RP, :],
                            a2_t[f0][:, RP * e:RP * e + RP],
                            a0[e][:, tch * TCH:(tch + 1) * TCH],
                            start=True, stop=True,
                            tile_position=(0, RP * el),
                            skip_group_check=True)
                for g in range(2):
                    ts = slice(tch * TCH, (tch + 1) * TCH)
                    if f0 == 0:
                        nc.vector.tensor_copy(t2acc[g][:, ts], pt2[g][:])
                    else:
                        nc.vector.tensor_tensor(t2acc[g][:, ts], pt2[g][:],
                                                t2acc[g][:, ts], op=ALU.add)
        pending_t2.clear()

    for f in range(n_fs):
        if f == 1:
            load_phase_b_weights()

        # base^T [128 f-rows, T] = W1s^T @ x^T, one weight load per
        # k-tile shared by both token chunks.
        pb = pbp.tile([P, T], F32, name="pb", tag="pb")
        w1f = w2p.tile([P, KT * P], BF16, name="w1f", tag="w1f", bufs=2)
        nc.scalar.dma_start(
            w1f[:].rearrange("p (k c) -> p k c", c=P),
            w1s[f * KT * P:(f + 1) * KT * P, :]
            .rearrange("(k p) c -> p k c", p=P))
        for k in range(KT):
            for tch in range(n_tch):
                nc.tensor.matmul(pb[:, tch * TCH:(tch + 1) * TCH],
                                 w1f[:, k * P:(k + 1) * P],
                                 xbf_t[k][:, tch * TCH:(tch + 1) * TCH],
                                 start=(k == 0), stop=(k == KT - 1),
                                 skip_group_check=True)

        base_sb = main.tile([P, T], BF16, name="base_sb", tag="base_sb",
                            bufs=2)
        nc.scalar.copy(base_sb[:], pb[:])

        flush_t2()

        a_t = [main.tile([P, T], BF16, name="a_sb", tag=f"a{e}", bufs=1)
               for e in range(E)]
        ca_t = []

        # 4 expert-pair groups: 2 row-packed l1 strips x 2 token chunks
        # (strip weights shared), then the base add via identity matmul,
        # then gelu straight out of PSUM.
        for pair in range(4):
            g = pair // 2
            els = [(2 * pair) % 4, (2 * pair) % 4 + 1]
            banks = {}
            for tch in range(n_tch):
                ts = slice(tch * TCH, (tch + 1) * TCH)
                for el in els:
                    pl = plp.tile([P, TCH], F32, name="pl", tag="pl")
                    rs = slice(RP * el, RP * el + RP)
                    nc.tensor.matmul(pl[:], b1r_t[g][rs, f * P:(f + 1) * P],
                                     t1_t[g][rs, ts],
                                     start=True, stop=False,
                                     tile_position=(RP * el, 0))
                    banks[(tch, el)] = pl
            for tch in range(n_tch):
                ts = slice(tch * TCH, (tch + 1) * TCH)
                for el in els:
                    nc.tensor.matmul(banks[(tch, el)][:], ident_bf[:],
                                     base_sb[:, ts], start=False, stop=True,
                                     skip_group_check=True)
            for tch in range(n_tch):
                ts = slice(tch * TCH, (tch + 1) * TCH)
                for el in els:
                    e = 4 * g + el
                    nc.scalar.activation(a_t[e][:, ts], banks[(tch, el)][:],
                                         AF.Gelu_apprx_tanh,
                                         bias=b1s_t[:, f:f + 1], scale=1.0)

        # ca = a * combine_e; mix = tree sum (4 adds DVE, 3 gpsimd)
        for e in range(E):
            ca = main.tile([P, T], BF16, name="ca_sb", tag=f"ca{e}", bufs=1)
            nc.vector.tensor_mul(ca[:], a_t[e][:], cbc_t[e][:])
            ca_t.append(ca)
        s01 = main.tile([P, T], BF16, name="s01", tag="s01", bufs=1)
        nc.vector.tensor_add(s01[:], ca_t[0][:], ca_t[1][:])
        s23 = main.tile([P, T], BF16, name="s23", tag="s23", bufs=1)
        nc.gpsimd.tensor_add(s23[:], ca_t[2][:], ca_t[3][:])
        s45 = main.tile([P, T], BF16, name="s45", tag="s45", bufs=1)
        nc.vector.tensor_add(s45[:], ca_t[4][:], ca_t[5][:])
        s67 = main.tile([P, T], BF16, name="s67", tag="s67", bufs=1)
        nc.gpsimd.tensor_add(s67[:], ca_t[6][:], ca_t[7][:])
        q0 = main.tile([P, T], BF16, name="q0", tag="q0", bufs=1)
        nc.vector.tensor_add(q0[:], s01[:], s23[:])
        q1 = main.tile([P, T], BF16, name="q1", tag="q1", bufs=1)
        nc.gpsimd.tensor_add(q1[:], s45[:], s67[:])
        mix_f = mixp.tile([P, T], BF16, name="mix", tag=f"mix{f}", bufs=1)
        nc.vector.tensor_add(mix_f[:], q0[:], q1[:])
        mix_all[f] = mix_f

        pending_t2.append((f, a_t))
    flush_t2()
    mainps.close()

    # t2 finalize: scale ranks by combine (t2 = t2' * w_e per rank row),
    # then overwrite row 16 (= e0 pad row) with csum for the b2 rank-1
    # term (core 0 carries b2 in b2rT row 16).
    for g in range(2):
        nc.vector.tensor_tensor(t2s[g][:], t2acc[g][:], cbcr_t[g][:],
                                op=ALU.mult)
    nc.sync.dma_start(t2s[0][R:R + 1, :], cbf[E:E + 1, :])

    # ---------------- W2 phase ----------------
    # out^T = W2s^T @ mix + B2pad^T @ t2pad; both token chunks share each
    # stationary W2 slice.
    with tc.tile_pool(name="o_ps", bufs=3, space="PSUM") as pop:
        for dm in range(n_dm):
            po = pop.tile([P, T], F32, name="po", tag="po")
            for f in range(n_fs):
                for tch in range(n_tch):
                    ts = slice(tch * TCH, (tch + 1) * TCH)
                    nc.tensor.matmul(po[:, ts],
                                     w2_t[f][:, dm * P:(dm + 1) * P],
                                     mix_all[f][:, ts],
                                     start=(f == 0), stop=False,
                                     skip_group_check=True)
            for g in range(2):
                for tch in range(n_tch):
                    ts = slice(tch * TCH, (tch + 1) * TCH)
                    nc.tensor.matmul(po[:, ts],
                                     b2r_t[g][:, dm * P:(dm + 1) * P],
                                     t2s[g][:, ts],
                                     start=False, stop=(g == 1),
                                     skip_group_check=True)
            o_sb = outp.tile([P, T], F32, name="o_sb", tag="o_sb")
            nc.scalar.copy(o_sb[:], po[:])
            nc.sync.dma_start(outT[dm * P:(dm + 1) * P, :], o_sb[:])

    ctx.close()


# --------------------------------------------------------------------------
# host-side sharding / gather
# --------------------------------------------------------------------------

def make_in_maps(hidden_states, Wr, br, W1, b1, W2, b2, A1, B1, A2, B2):
    """Build the 8 per-core input dicts from full fp32 inputs."""
    hidden_states, Wr, br, W1, b1, W2, b2, A1, B1, A2, B2 = (
        np.asarray(a) for a in
        (hidden_states, Wr, br, W1, b1, W2, b2, A1, B1, A2, B2))
    bf16 = ml_dtypes.bfloat16
    T = hidden_states.shape[0] * hidden_states.shape[1]
    x = np.ascontiguousarray(hidden_states.reshape(T, D).astype(np.float32))
    xT = np.ascontiguousarray(x.T)                      # [D, T]
    xTb = xT.astype(bf16)
    xTl = (xT - xTb.astype(np.float32)).astype(bf16)    # router lo part

    wrh = Wr.astype(bf16)
    wrl = (Wr.astype(np.float32) - wrh.astype(np.float32)).astype(bf16)
    brv = br.astype(np.float32).reshape(E, 1)

    a1T = np.zeros((D, P), dtype=bf16)
    for e in range(E):
        a1T[:, R * e:R * e + R] = A1[e].T.astype(bf16)  # A1[e] is [R, D]

    in_maps = []
    for c in range(NCORES):
        s = slice(c * FS, (c + 1) * FS)
        w1s = np.ascontiguousarray(
            W1[:, s].reshape(D // P, P, FS // P, P).transpose(2, 0, 1, 3)
            .reshape(-1, P)).astype(bf16)
        w2s = np.ascontiguousarray(W2[s, :]).astype(bf16)

        b1rT = np.zeros((2 * P, FS), dtype=bf16)
        a2sT = np.zeros((FS, 2 * P), dtype=bf16)
        for e in range(E):
            b1rT[RP * e:RP * e + R, :] = (B1[e, s, :].T * SCALING).astype(bf16)
            a2sT[:, RP * e:RP * e + R] = A2[e, :, s].T.astype(bf16)

        b2rT = np.zeros((2 * P, D), dtype=bf16)
        for e in range(E):
            b2rT[RP * e:RP * e + R, :] = (B2[e].T * SCALING).astype(bf16)
        if c == 0:
            b2rT[R, :] = b2.astype(np.float32).astype(bf16)

        b1sM = np.ascontiguousarray(
            b1[s].astype(np.float32).reshape(FS // P, P).T)   # [P, FS//P]

        in_maps.append(dict(
            xTb=xTb, xTl=xTl, w1s=w1s, w2s=w2s, a1T=a1T,
            b1rT=b1rT, a2sT=a2sT, b2rT=b2rT,
            wrh=wrh, wrl=wrl, brv=brv, b1sM=b1sM,
            idf=np.eye(P, dtype=np.float32),
            idb=np.eye(P, dtype=np.float32).astype(bf16),
        ))
    return in_maps


_nc_cache = {}


def _get_nc(T):
    if T not in _nc_cache:
        _nc_cache[T] = build_nc(T)
    return _nc_cache[T]


_last_results = None


def _ensure_ntff_hook():
    """Install the axon NTFF profiling hook if the image's antenv lacks
    axon_hooks (needed for trace=True timing under axon)."""
    import types
    try:
        import antenv
        if "antenv.axon_hooks" not in sys.modules:
            mod = types.ModuleType("antenv.axon_hooks")
            mod._hook = None

            def set_axon_ntff_profile_hook(h):
                mod._hook = h

            def get_axon_ntff_profile_hook():
                return mod._hook

            mod.set_axon_ntff_profile_hook = set_axon_ntff_profile_hook
            mod.get_axon_ntff_profile_hook = get_axon_ntff_profile_hook
            sys.modules["antenv.axon_hooks"] = mod
            antenv.axon_hooks = mod
        hooks = sys.modules["antenv.axon_hooks"]
        if hooks.get_axon_ntff_profile_hook() is None:
            if "/root/.axon_site" not in sys.path:
                sys.path.insert(0, "/root/.axon_site")
            from trn_agent_boot.trn_boot import _ntff_profile_via_ctypes
            hooks.set_axon_ntff_profile_hook(
                _ntff_profile_via_ctypes("/opt/axon/libaxon_pjrt.so"))
    except Exception as e:  # profiling is best-effort
        print(f"ntff hook setup failed: {e}", file=sys.stderr)


def kernel(hidden_states, Wr, br, W1, b1, W2, b2, A1, B1, A2, B2,
           trace=False):
    global _last_results
    from concourse.bass_utils import run_bass_kernel_spmd
    if trace:
        _ensure_ntff_hook()

    B, S, _ = hidden_states.shape
    T = B * S
    nc = _get_nc(T)
    in_maps = make_in_maps(hidden_states, Wr, br, W1, b1, W2, b2,
                           A1, B1, A2, B2)
    tmpdir = os.environ.get("KERNEL_TRACE_DIR") or None
    if tmpdir:
        os.makedirs(tmpdir, exist_ok=True)
    res = run_bass_kernel_spmd(nc, in_maps, list(range(NCORES)), trace=trace,
                               tmpdir=tmpdir)
    _last_results = res
    out = np.zeros((T, D), dtype=np.float64)
    for c in range(NCORES):
        out += res.results[c]["outT"].astype(np.float64).T
    return out.astype(np.float32).reshape(B, S, D)


# revision 17
# speedup vs baseline: 1.2645x; 1.2645x over previous
"""Trainium2 Bass kernel for the nn_Experts MoE-LoRA problem.

Computes, for x = hidden_states.reshape(T, D):
    probs   = softmax(x @ Wr + br)
    w, idx  = top2(probs); combine[t,e] = w if e selected else 0
    base    = x @ W1                     (b1 folded into the gelu bias)
    t1      = einsum('td,erd->ter', x, A1)
    l1      = einsum('ter,efr->tef', t1, B1) * 2.0
    a       = gelu_tanh(base[:,None,:] + b1 + l1)
    ca      = a * combine[:,:,None]
    mix     = ca.sum(1)
    t2      = einsum('tef,erf->ter', ca, A2)
    l2      = einsum('ter,edr->td', t2, B2) * 2.0
    out     = mix @ W2 + combine.sum(-1,keepdims) * b2 + l2

Sharding: the F=8192 ff dimension is split across the 8 cores (Fs=1024
per core).  Each core holds the full token set and all 8 experts'
LoRA factors restricted to its F-slice, and produces a partial
out^T = W2s^T @ mix_s^T + l2_partial, which the host sums over cores.

v3 design (vs the 295us baseline):
  - router logits via a 3-term bf16 split (hi*hi + lo*hi + hi*lo)
    instead of fp32 LOW_HIGH matmuls: 0 top-2 flips on this data with
    a ~49x logit-gap margin, and no 8.4MB fp32 x load (xlo reuses the
    fp32 byte budget).  The 48 (term,k) tile-matmuls per chunk are
    dealt round-robin onto 4 col-packed PE sub-chains (step-major
    emission so different col groups stream concurrently) and reduced
    with DVE adds + one scalar_tensor_tensor that also applies br.
  - base GEMM shares each W1 weight tile across both 512-token chunks
    (LDW + 2 matmuls), and its 16 k-steps are software-pipelined into
    the previous f's strip/ident groups so the PE stream stays dense
    while ACT drains the gelus.
  - t2 is computed from a (not ca), accumulated over f in SBUF, and
    scaled by combine once at the end in rank space, so the whole PE
    pipeline is router-independent.
  - all DMAs ride the sync queue (gpsimd for the combine broadcasts):
    engine-queue dma triggers head-of-line block that engine.
  - router weights/a1/a2 are pre-packed on host into the on-chip
    layout so their loads are single contiguous DMAs.
  - PSUM: base pair 2 banks + 4 strip banks + 2 transient t2 = 8.
"""

import os
import sys

for _p in ("/opt/trn_rl_repo", os.path.join(os.path.dirname(os.path.abspath(__file__)))):
    if _p not in sys.path:
        sys.path.insert(0, _p)

import numpy as np
import ml_dtypes

import concourse.bass as bass
import concourse.mybir as mybir
import concourse.tile as tile
from concourse import bacc

BF16 = mybir.dt.bfloat16
F32 = mybir.dt.float32
AF = mybir.ActivationFunctionType
ALU = mybir.AluOpType
AX = mybir.AxisListType

E = 8      # experts
K = 2      # top-k
D = 2048   # hidden
F = 8192   # ff dim (full)
R = 16     # lora rank
RP = 32    # padded rank (32-aligned for PE row/col strips)
SCALING = 2.0
NCORES = 8
FS = F // NCORES   # per-core ff slice = 1024
P = 128
TCH = 512          # token chunk (one PSUM bank of fp32)


# --------------------------------------------------------------------------
# device program
# --------------------------------------------------------------------------

def build_nc(T: int) -> bass.Bass:
    assert T % TCH == 0
    n_tch = T // TCH          # 2
    n_mt = T // P             # token tiles (8)
    KT = D // P               # contraction tiles over D = 16

    nc = bacc.Bacc("TRN2", target_bir_lowering=False, debug=False,
                   num_devices=NCORES)

    # ---- DRAM parameters (per-core data; pre-packed on host) ----
    cstage = nc.dram_tensor("cstage", [9, T], BF16).ap()
    xTb = nc.dram_tensor("xTb", [D, T], BF16, kind="ExternalInput").ap()
    xTl = nc.dram_tensor("xTl", [D, T], BF16, kind="ExternalInput").ap()
    w1s = nc.dram_tensor("w1s", [(FS // P) * KT * P, P], BF16,
                         kind="ExternalInput").ap()
    w2s = nc.dram_tensor("w2s", [FS, D], BF16, kind="ExternalInput").ap()
    a1p = nc.dram_tensor("a1p", [P, KT * P], BF16, kind="ExternalInput").ap()
    b1rT = nc.dram_tensor("b1rT", [2 * P, FS], BF16, kind="ExternalInput").ap()
    a2p = nc.dram_tensor("a2p", [P, (FS // P) * 2 * P], BF16,
                         kind="ExternalInput").ap()
    b2rT = nc.dram_tensor("b2rT", [2 * P, D], BF16, kind="ExternalInput").ap()
    wrp = nc.dram_tensor("wrp", [P, 2 * KT * E], BF16,
                         kind="ExternalInput").ap()
    brv = nc.dram_tensor("brv", [E, 1], F32, kind="ExternalInput").ap()
    b1sM = nc.dram_tensor("b1sM", [P, FS // P], F32, kind="ExternalInput").ap()
    idf = nc.dram_tensor("idf", [P, P], F32, kind="ExternalInput").ap()
    idb = nc.dram_tensor("idb", [P, P], BF16, kind="ExternalInput").ap()
    outT = nc.dram_tensor("outT", [D, T], F32, kind="ExternalOutput").ap()

    with tile.TileContext(nc) as tc:
        _emit(tc, T, n_tch, n_mt, KT,
              xTb, xTl, w1s, w2s, a1p, b1rT, a2p, b2rT,
              wrp, brv, b1sM, outT, cstage, idf, idb)
    nc.compile()
    return nc


def _emit(tc, T, n_tch, n_mt, KT,
          xTb, xTl, w1s, w2s, a1p, b1rT, a2p, b2rT,
          wrp, brv, b1sM, outT, cstage, idf, idb):
    nc = tc.nc
    from contextlib import ExitStack
    ctx = ExitStack()

    # ---------------- resident pool; small consts first ----------------
    resid = ctx.enter_context(tc.tile_pool(name="resid", bufs=1))

    ident = resid.tile([P, P], F32, name="ident", tag="ident")
    nc.sync.dma_start(ident[:], idf[:, :])
    ident_bf = resid.tile([P, P], BF16, name="ident_bf", tag="ident_bf")
    nc.sync.dma_start(ident_bf[:], idb[:, :])
    wrp_t = resid.tile([P, 2 * KT * E], BF16, name="wrp_t", tag="wrp_t")
    nc.sync.dma_start(wrp_t[:], wrp[:, :])
    wh_k = [wrp_t[:, k * E:(k + 1) * E] for k in range(KT)]
    wl_k = [wrp_t[:, KT * E + k * E:KT * E + (k + 1) * E] for k in range(KT)]
    brv_t = resid.tile([E, 1], F32, name="brv_t", tag="brv_t")
    nc.sync.dma_start(brv_t[:], brv[:, :])
    b1s_t = resid.tile([P, FS // P], F32, name="b1s_t", tag="b1s_t")
    nc.sync.dma_start(b1s_t[:], b1sM[:, :])

    # x hi (bf16) resident, chunk-major loads: chunk c covers tokens
    # [c*TCH,(c+1)*TCH) for all 16 k-tiles.
    xbf_all = resid.tile([P, KT * T], BF16, name="xbf_all", tag="xbf_all")
    xbf_t = [xbf_all[:, k * T:(k + 1) * T] for k in range(KT)]

    rtr = ExitStack()
    xlo_p = rtr.enter_context(tc.tile_pool(name="xlo_p", bufs=2))
    xlo_c = []

    a1_all = resid.tile([P, KT * P], BF16, name="a1_all", tag="a1_all")
    a1_t = [a1_all[:, k * P:(k + 1) * P] for k in range(KT)]

    def load_x_chunk(c):
        ts = slice(c * TCH, (c + 1) * TCH)
        nc.sync.dma_start(
            xbf_all[:].rearrange("p (k t) -> p k t", t=T)[:, :, ts],
            xTb[:, ts].rearrange("(k p) t -> p k t", p=P))
        t = xlo_p.tile([P, KT * TCH], BF16, name="xlo", tag=f"xlo{c}", bufs=1)
        nc.sync.dma_start(
            t[:].rearrange("p (k t) -> p k t", t=TCH),
            xTl[:, ts].rearrange("(k p) t -> p k t", p=P))
        xlo_c.append(t)

    load_x_chunk(0)
    nc.sync.dma_start(a1_all[:], a1p[:, :])
    load_x_chunk(1)

    b1r_t = []
    for g in range(2):
        t = resid.tile([P, FS], BF16, name=f"b1r{g}", tag=f"b1r{g}")
        nc.sync.dma_start(t[:], b1rT[g * P:(g + 1) * P, :])
        b1r_t.append(t)
    a2_all = resid.tile([P, (FS // P) * 2 * P], BF16, name="a2_all",
                        tag="a2_all")
    nc.sync.dma_start(a2_all[:], a2p[:, :])
    a2_t = [a2_all[:, f * 2 * P:(f + 1) * 2 * P] for f in range(FS // P)]

    # combine tiles (filled by router)
    cbf = resid.tile([9, T], BF16, name="cbf", tag="cbf")
    cbc_t = [resid.tile([P, T], BF16, name=f"cbc{e}", tag=f"cbc{e}")
             for e in range(E)]
    cbcr_t = [resid.tile([P, T], BF16, name=f"cbcr{g}", tag=f"cbcr{g}")
              for g in range(2)]

    t1_t = [resid.tile([P, T], BF16, name=f"t1_{g}", tag=f"t1_{g}")
            for g in range(2)]
    t2acc = [resid.tile([P, T], F32, name=f"t2a{g}", tag=f"t2a{g}")
             for g in range(2)]
    t2s = [resid.tile([P, T], BF16, name=f"t2s{g}", tag=f"t2s{g}")
           for g in range(2)]
    mix_all = [None] * (FS // P)

    # ---------------- PE warmup (HAM) ----------------
    with tc.tile_pool(name="warm_ps", bufs=1, space="PSUM") as wps:
        wt = wps.tile([P, P], F32, name="warm", tag="warm")
        for _ in range(30):
            nc.tensor.transpose(wt[:], ident[:], ident[:])

    # ---------------- router ----------------
    # logits^T [E, T] = Whi^T xhi + Whi^T xlo + Wlo^T xhi.  The 48
    # (term,k) tile-matmuls per chunk are dealt round-robin onto 4
    # col-packed sub-chains (psum partitions 32j..32j+8) and emitted
    # step-major so the four col groups stream concurrently.
    rsb = rtr.enter_context(tc.tile_pool(name="router_sb", bufs=3))
    lgT = rsb.tile([E, T], F32, name="lgT", tag="lgT", bufs=1)
    rps = rtr.enter_context(tc.tile_pool(name="router_ps", bufs=2,
                                         space="PSUM"))

    def router_chain(c):
        ts = slice(c * TCH, (c + 1) * TCH)
        plg = rps.tile([P, TCH], F32, name="plg", tag="plg")
        terms = ([(wh_k[k], xbf_t[k][:, ts]) for k in range(KT)] +
                 [(wh_k[k], xlo_c[c][:, k * TCH:(k + 1) * TCH])
                  for k in range(KT)] +
                 [(wl_k[k], xbf_t[k][:, ts]) for k in range(KT)])
        n = len(terms)
        for i, (w, xx) in enumerate(terms):
            j = i % 4
            nc.tensor.matmul(plg[RP * j:RP * j + E, :], w, xx,
                             start=(i < 4), stop=(i >= n - 4),
                             tile_position=(0, RP * j),
                             skip_group_check=True)
        # DVE may read only one PSUM operand per op: route two
        # sub-accumulators through ACT copies first.
        cA = rsb.tile([E, TCH], F32, name="rcA", tag="rcA")
        nc.scalar.copy(cA[:], plg[0:E, :])
        cB = rsb.tile([E, TCH], F32, name="rcB", tag="rcB")
        nc.scalar.copy(cB[:], plg[2 * RP:2 * RP + E, :])
        s1 = rsb.tile([E, TCH], F32, name="rs1", tag="rs1")
        nc.vector.tensor_tensor(s1[:], cA[:], plg[RP:RP + E, :], op=ALU.add)
        s2 = rsb.tile([E, TCH], F32, name="rs2", tag="rs2")
        nc.vector.tensor_tensor(s2[:], cB[:], plg[3 * RP:3 * RP + E, :],
                                op=ALU.add)
        nc.vector.scalar_tensor_tensor(lgT[:, ts], s1[:], brv_t[:, 0:1],
                                       s2[:], op0=ALU.add, op1=ALU.add)

    def router_transposes(c):
        # batch the 4 logit transposes of a chunk into one PSUM bank so
        # the PE queue isn't blocked per-m-tile on the DVE softmax.
        prb = rps.tile([P, 4 * E], F32, name="prb", tag="prb")
        for mi in range(4):
            m = 4 * c + mi
            nc.tensor.transpose(prb[:, mi * E:(mi + 1) * E],
                                lgT[:, m * P:(m + 1) * P], ident[:E, :E])
        return prb

    def router_softmax(c, prb):
        # per-m-tile softmax + top-2 on DVE; combine rows come back to
        # E-major via one batched PE transpose.
        comb4 = rsb.tile([P, 4 * (E + 1)], F32, name="comb4", tag="comb4")
        for mi in range(4):
            pr = prb[:, mi * E:(mi + 1) * E]
            comb = comb4[:, mi * (E + 1):(mi + 1) * (E + 1)]
            negmax = rsb.tile([P, 1], F32, name="negmax", tag="negmax")
            nc.vector.tensor_reduce(negmax[:], pr, axis=AX.X, op=ALU.max,
                                    negate=True)
            pexp = rsb.tile([P, E], F32, name="pexp", tag="pexp")
            nc.scalar.activation(pexp[:], pr, AF.Exp, bias=negmax[:, 0:1],
                                 scale=1.0)
            ssum = rsb.tile([P, 1], F32, name="ssum", tag="ssum")
            nc.vector.tensor_reduce(ssum[:], pexp[:], axis=AX.X, op=ALU.add)
            rsum = rsb.tile([P, 1], F32, name="rsum", tag="rsum")
            nc.vector.reciprocal(rsum[:], ssum[:])
            probs = rsb.tile([P, E], F32, name="probs", tag="probs")
            nc.vector.tensor_scalar_mul(probs[:], pexp[:], rsum[:, 0:1])
            m1 = rsb.tile([P, 1], F32, name="m1", tag="m1")
            nc.vector.tensor_reduce(m1[:], probs[:], axis=AX.X, op=ALU.max)
            mask1 = rsb.tile([P, E], F32, name="mask1", tag="mask1")
            nc.vector.tensor_scalar(mask1[:], probs[:], m1[:, 0:1], None,
                                    op0=ALU.is_ge)
            pm = rsb.tile([P, E], F32, name="pm", tag="pm")
            nc.vector.scalar_tensor_tensor(pm[:], mask1[:], -2.0, probs[:],
                                           op0=ALU.mult, op1=ALU.add)
            m2 = rsb.tile([P, 1], F32, name="m2", tag="m2")
            nc.vector.tensor_reduce(m2[:], pm[:], axis=AX.X, op=ALU.max)
            mask2 = rsb.tile([P, E], F32, name="mask2", tag="mask2")
            nc.vector.tensor_scalar(mask2[:], probs[:], m2[:, 0:1], None,
                                    op0=ALU.is_ge)
            nc.vector.tensor_tensor(comb[0:P, 0:E], probs[:], mask2[:],
                                    op=ALU.mult)
            nc.vector.tensor_reduce(comb[0:P, E:E + 1], comb[0:P, 0:E],
                                    axis=AX.X, op=ALU.add)
        return comb4

    def router_combine_out(c, comb4):
        ptb = rps.tile([E + 1, 4 * P], F32, name="ptb", tag="ptb")
        for mi in range(4):
            nc.tensor.transpose(ptb[:, mi * P:(mi + 1) * P],
                                comb4[:, mi * (E + 1):(mi + 1) * (E + 1)],
                                ident[:])
        ts = slice(c * TCH, (c + 1) * TCH)
        nc.scalar.copy(cbf[:, ts], ptb[:])
        # stage through DRAM (SBUF-source partition-broadcast DMA is
        # rejected); per chunk, so chunk-0 rows land early.
        nc.gpsimd.dma_start(cstage[:, ts], cbf[:, ts])
        for e in range(E):
            nc.gpsimd.dma_start(cbc_t[e][:, ts],
                                cstage[e:e + 1, ts].to_broadcast([P, TCH]))
        for e in range(E):
            g, el = divmod(e, 4)
            nc.gpsimd.dma_start(cbcr_t[g][RP * el:RP * el + RP, ts],
                                cstage[e:e + 1, ts].to_broadcast([RP, TCH]))

    # ---------------- t1 ----------------
    t1un = resid.tile([P, T], BF16, name="t1un", tag="t1un")

    def t1_chunk(c):
        ts = slice(c * TCH, (c + 1) * TCH)
        pt1 = rps.tile([P, TCH], F32, name="pt1", tag="pt1")
        for k in range(KT):
            nc.tensor.matmul(pt1[:], a1_t[k][:], xbf_t[k][:, ts],
                             start=(k == 0), stop=(k == KT - 1))
        nc.scalar.copy(t1un[:, ts], pt1[:])

    def t1_spread():
        nc.vector.memset(t1_t[0][:], 0.0)
        nc.vector.memset(t1_t[1][:], 0.0)
        for e in range(E):
            g, el = divmod(e, 4)
            nc.sync.dma_start(t1_t[g][RP * el:RP * el + R, :],
                              t1un[R * e:R * e + R, :])

    # PE emission order interleaves router chunks with t1 so the PE is
    # never parked behind a DMA it doesn't need yet; the DVE softmax of
    # chunk c runs while the PE does t1 / the next chain.
    router_chain(0)
    prb0 = router_transposes(0)
    t1_chunk(0)
    comb40 = router_softmax(0, prb0)
    router_combine_out(0, comb40)
    router_chain(1)
    prb1 = router_transposes(1)
    t1_chunk(1)
    comb41 = router_softmax(1, prb1)
    router_combine_out(1, comb41)
    t1_spread()
    rtr.close()

    # ---------------- main pipeline ----------------
    n_fs = FS // P     # 8 f-tiles per core
    n_dm = D // P      # 16 output d-tiles

    main = ctx.enter_context(tc.tile_pool(name="main_sb", bufs=3))
    mixp = ctx.enter_context(tc.tile_pool(name="mix_sb", bufs=2))
    w2p = ctx.enter_context(tc.tile_pool(name="w2_sb", bufs=4))
    outp = ctx.enter_context(tc.tile_pool(name="out_sb", bufs=2))

    w2_t = []
    b2r_t = []

    def load_phase_b_weights():
        for f in range(n_fs):
            t = w2p.tile([P, D], BF16, name=f"w2_{f}", tag=f"w2_{f}", bufs=1)
            nc.sync.dma_start(t[:], w2s[f * P:(f + 1) * P, :])
            w2_t.append(t)
        for g in range(2):
            t = resid.tile([P, D], BF16, name=f"b2r{g}", tag=f"b2r{g}")
            nc.sync.dma_start(t[:], b2rT[g * P:(g + 1) * P, :])
            b2r_t.append(t)

    mainps = ExitStack()
    pbp = mainps.enter_context(tc.tile_pool(name="base_ps", bufs=1,
                                            space="PSUM"))
    plp = mainps.enter_context(tc.tile_pool(name="l1_ps", bufs=4,
                                            space="PSUM"))
    pt2p = mainps.enter_context(tc.tile_pool(name="t2_ps", bufs=1,
                                             space="PSUM"))

    def load_w1f(f):
        w1f = w2p.tile([P, KT * P], BF16, name="w1f", tag="w1f", bufs=2)
        nc.sync.dma_start(
            w1f[:].rearrange("p (k c) -> p k c", c=P),
            w1s[f * KT * P:(f + 1) * KT * P, :]
            .rearrange("(k p) c -> p k c", p=P))
        return w1f

    def base_steps(w1f, pb):
        # 16 k-steps; each = one W1 weight load shared by both chunks.
        for k in range(KT):
            for tch in range(n_tch):
                nc.tensor.matmul(pb[:, tch * TCH:(tch + 1) * TCH],
                                 w1f[:, k * P:(k + 1) * P],
                                 xbf_t[k][:, tch * TCH:(tch + 1) * TCH],
                                 start=(k == 0), stop=(k == KT - 1),
                                 skip_group_check=True)
            yield

    # t2 strips consume a(f) and are emitted during f+1, one (g,tch)
    # quarter per pair group, accumulating into SBUF.
    def t2_quarter(f0, a0, q):
        g, tch = q // 2, q % 2
        ts = slice(tch * TCH, (tch + 1) * TCH)
        pt2 = pt2p.tile([P, TCH], F32, name="pt2", tag=f"pt2_{tch}")
        for el in range(4):
            e = 4 * g + el
            nc.tensor.matmul(pt2[RP * el:RP * el + RP, :],
                             a2_t[f0][:, RP * e:RP * e + RP],
                             a0[e][:, ts], start=True, stop=True,
                             tile_position=(0, RP * el),
                             skip_group_check=True)
        if f0 == 0:
            nc.vector.tensor_copy(t2acc[g][:, ts], pt2[:])
        else:
            nc.vector.tensor_tensor(t2acc[g][:, ts], pt2[:],
                                    t2acc[g][:, ts], op=ALU.add)

    # software pipeline: base chain for f+1 is interleaved into f's
    # strip/ident groups (4 k-steps per pair group).
    w1f_cur = load_w1f(0)
    pb = pbp.tile([P, T], F32, name="pb", tag="pb")
    for _ in base_steps(w1f_cur, pb):
        pass
    base_sb = main.tile([P, T], BF16, name="base_sb", tag="base_sb", bufs=2)
    nc.scalar.copy(base_sb[:], pb[:])

    prev = None  # (f-1, a tiles)
    for f in range(n_fs):
        if f == n_fs - 1:
            load_phase_b_weights()
        if f + 1 < n_fs:
            w1f_next = load_w1f(f + 1)
            pb = pbp.tile([P, T], F32, name="pb", tag="pb")
            filler = base_steps(w1f_next, pb)
        else:
            filler = iter(())

        a_t = [main.tile([P, T], BF16, name="a_sb", tag=f"a{e}", bufs=2)
               for e in range(E)]

        for pair in range(4):
            g = pair // 2
            els = [(2 * pair) % 4, (2 * pair) % 4 + 1]
            banks = {}
            for tch in range(n_tch):
                ts = slice(tch * TCH, (tch + 1) * TCH)
                for el in els:
                    pl = plp.tile([P, TCH], F32, name="pl", tag="pl")
                    rs = slice(RP * el, RP * el + RP)
                    nc.tensor.matmul(pl[:], b1r_t[g][rs, f * P:(f + 1) * P],
                                     t1_t[g][rs, ts],
                                     start=True, stop=False,
                                     tile_position=(RP * el, 0))
                    banks[(tch, el)] = pl
            for tch in range(n_tch):
                ts = slice(tch * TCH, (tch + 1) * TCH)
                for el in els:
                    nc.tensor.matmul(banks[(tch, el)][:], ident_bf[:],
                                     base_sb[:, ts], start=False, stop=True,
                                     skip_group_check=True)
            # 4 base k-steps of f+1 keep the PE fed while ACT drains
            for _ in range(4):
                next(filler, None)
            # one t2 quarter of f-1
            if prev is not None:
                t2_quarter(prev[0], prev[1], pair)
            for tch in range(n_tch):
                ts = slice(tch * TCH, (tch + 1) * TCH)
                for el in els:
                    e = 4 * g + el
                    nc.scalar.activation(a_t[e][:, ts], banks[(tch, el)][:],
                                         AF.Gelu_apprx_tanh,
                                         bias=b1s_t[:, f:f + 1], scale=1.0)

        for _ in filler:
            pass
        if f + 1 < n_fs:
            base_sb = main.tile([P, T], BF16, name="base_sb", tag="base_sb",
                                bufs=2)
            nc.scalar.copy(base_sb[:], pb[:])
            w1f_cur = w1f_next

        # ca = a * combine_e; mix = tree sum in two 4-expert waves that
        # share the same 4 ca buffers (DVE + gpsimd)
        def ca_wave(e0):
            cas = []
            for i, e in enumerate(range(e0, e0 + 4)):
                ca = main.tile([P, T], BF16, name="ca_sb", tag=f"ca{i}",
                               bufs=1)
                nc.vector.tensor_mul(ca[:], a_t[e][:], cbc_t[e][:])
                cas.append(ca)
            sA = main.tile([P, T], BF16, name="sA", tag="sA", bufs=1)
            nc.vector.tensor_add(sA[:], cas[0][:], cas[1][:])
            sB = main.tile([P, T], BF16, name="sB", tag="sB", bufs=1)
            nc.gpsimd.tensor_add(sB[:], cas[2][:], cas[3][:])
            return sA, sB
        sA0, sB0 = ca_wave(0)
        mA = main.tile([P, T], BF16, name="mA", tag="mA", bufs=1)
        nc.vector.tensor_add(mA[:], sA0[:], sB0[:])
        sA1, sB1 = ca_wave(4)
        mB = main.tile([P, T], BF16, name="mB", tag="mB", bufs=1)
        nc.gpsimd.tensor_add(mB[:], sA1[:], sB1[:])
        mix_f = mixp.tile([P, T], BF16, name="mix", tag=f"mix{f}", bufs=1)
        nc.vector.tensor_add(mix_f[:], mA[:], mB[:])
        mix_all[f] = mix_f

        prev = (f, a_t)

    # last f's t2 quarters
    for q in range(4):
        t2_quarter(prev[0], prev[1], q)
    mainps.close()

    # t2 finalize: scale ranks by combine, then overwrite row 16 (= e0
    # pad row) with csum for the b2 rank-1 term (core 0 carries b2 in
    # b2rT row 16).
    for g in range(2):
        nc.vector.tensor_tensor(t2s[g][:], t2acc[g][:], cbcr_t[g][:],
                                op=ALU.mult)
    nc.sync.dma_start(t2s[0][R:R + 1, :], cbf[E:E + 1, :])

    # ---------------- W2 phase ----------------
    with tc.tile_pool(name="o_ps", bufs=3, space="PSUM") as pop:
        for dm in range(n_dm):
            po = pop.tile([P, T], F32, name="po", tag="po")
            for f in range(n_fs):
                for tch in range(n_tch):
                    ts = slice(tch * TCH, (tch + 1) * TCH)
                    nc.tensor.matmul(po[:, ts],
                                     w2_t[f][:, dm * P:(dm + 1) * P],
                                     mix_all[f][:, ts],
                                     start=(f == 0), stop=False,
                                     skip_group_check=True)
            for g in range(2):
                for tch in range(n_tch):
                    ts = slice(tch * TCH, (tch + 1) * TCH)
                    nc.tensor.matmul(po[:, ts],
                                     b2r_t[g][:, dm * P:(dm + 1) * P],
                                     t2s[g][:, ts],
                                     start=False, stop=(g == 1),
                                     skip_group_check=True)
            for tch in range(n_tch):
                ts = slice(tch * TCH, (tch + 1) * TCH)
                o_sb = outp.tile([P, TCH], F32, name="o_sb", tag="o_sb",
                                 bufs=3)
                nc.scalar.copy(o_sb[:], po[:, ts])
                nc.sync.dma_start(outT[dm * P:(dm + 1) * P, ts], o_sb[:])

    ctx.close()


# --------------------------------------------------------------------------
# host-side sharding / gather
# --------------------------------------------------------------------------

def make_in_maps(hidden_states, Wr, br, W1, b1, W2, b2, A1, B1, A2, B2):
    """Build the 8 per-core input dicts from full fp32 inputs."""
    hidden_states, Wr, br, W1, b1, W2, b2, A1, B1, A2, B2 = (
        np.asarray(a) for a in
        (hidden_states, Wr, br, W1, b1, W2, b2, A1, B1, A2, B2))
    bf16 = ml_dtypes.bfloat16
    T = hidden_states.shape[0] * hidden_states.shape[1]
    x = np.ascontiguousarray(hidden_states.reshape(T, D).astype(np.float32))
    xT = np.ascontiguousarray(x.T)                      # [D, T]
    xTb = xT.astype(bf16)
    xTl = (xT - xTb.astype(np.float32)).astype(bf16)    # router lo part

    # router weights packed to the on-chip layout [P, 2*KT*E]
    KT = D // P
    wrh = Wr.astype(bf16).astype(np.float32)
    wrl = (Wr.astype(np.float32) - wrh)
    wrp = np.zeros((P, 2 * KT * E), dtype=bf16)
    wrp[:, :KT * E] = wrh.reshape(KT, P, E).transpose(1, 0, 2).reshape(P, -1)
    wrp[:, KT * E:] = wrl.reshape(KT, P, E).transpose(1, 0, 2).reshape(P, -1)
    brv = br.astype(np.float32).reshape(E, 1)

    # A1 packed to [P, KT*P]: block k holds A1 rows for d in [128k,128k+128)
    a1T = np.zeros((D, P), dtype=np.float32)
    for e in range(E):
        a1T[:, R * e:R * e + R] = A1[e].T               # A1[e] is [R, D]
    a1p = np.ascontiguousarray(
        a1T.reshape(KT, P, P).transpose(1, 0, 2).reshape(P, -1)).astype(bf16)

    in_maps = []
    for c in range(NCORES):
        s = slice(c * FS, (c + 1) * FS)
        w1s = np.ascontiguousarray(
            W1[:, s].reshape(D // P, P, FS // P, P).transpose(2, 0, 1, 3)
            .reshape(-1, P)).astype(bf16)
        w2s = np.ascontiguousarray(W2[s, :]).astype(bf16)

        b1rT = np.zeros((2 * P, FS), dtype=bf16)
        a2sT = np.zeros((FS, 2 * P), dtype=np.float32)
        for e in range(E):
            b1rT[RP * e:RP * e + R, :] = (B1[e, s, :].T * SCALING).astype(bf16)
            a2sT[:, RP * e:RP * e + R] = A2[e, :, s].T
        # A2 packed to [P, (FS//P)*2P]: block f holds rows [128f,128f+128)
        a2p = np.ascontiguousarray(
            a2sT.reshape(FS // P, P, 2 * P).transpose(1, 0, 2)
            .reshape(P, -1)).astype(bf16)

        b2rT = np.zeros((2 * P, D), dtype=bf16)
        for e in range(E):
            b2rT[RP * e:RP * e + R, :] = (B2[e].T * SCALING).astype(bf16)
        if c == 0:
            b2rT[R, :] = b2.astype(np.float32).astype(bf16)

        b1sM = np.ascontiguousarray(
            b1[s].astype(np.float32).reshape(FS // P, P).T)   # [P, FS//P]

        in_maps.append(dict(
            xTb=xTb, xTl=xTl, w1s=w1s, w2s=w2s, a1p=a1p,
            b1rT=b1rT, a2p=a2p, b2rT=b2rT,
            wrp=wrp, brv=brv, b1sM=b1sM,
            idf=np.eye(P, dtype=np.float32),
            idb=np.eye(P, dtype=np.float32).astype(bf16),
        ))
    return in_maps


_nc_cache = {}


def _get_nc(T):
    if T not in _nc_cache:
        _nc_cache[T] = build_nc(T)
    return _nc_cache[T]


_last_results = None


def _ensure_ntff_hook():
    """Install the axon NTFF profiling hook if the image's antenv lacks
    axon_hooks (needed for trace=True timing under axon)."""
    import types
    try:
        import antenv
        if "antenv.axon_hooks" not in sys.modules:
            mod = types.ModuleType("antenv.axon_hooks")
            mod._hook = None

            def set_axon_ntff_profile_hook(h):
                mod._hook = h

            def get_axon_ntff_profile_hook():
                return mod._hook

            mod.set_axon_ntff_profile_hook = set_axon_ntff_profile_hook
            mod.get_axon_ntff_profile_hook = get_axon_ntff_profile_hook
            sys.modules["antenv.axon_hooks"] = mod
            antenv.axon_hooks = mod
        hooks = sys.modules["antenv.axon_hooks"]
        if hooks.get_axon_ntff_profile_hook() is None:
            if "/root/.axon_site" not in sys.path:
                sys.path.insert(0, "/root/.axon_site")
            from trn_agent_boot.trn_boot import _ntff_profile_via_ctypes
            hooks.set_axon_ntff_profile_hook(
                _ntff_profile_via_ctypes("/opt/axon/libaxon_pjrt.so"))
    except Exception as e:  # profiling is best-effort
        print(f"ntff hook setup failed: {e}", file=sys.stderr)


def kernel(hidden_states, Wr, br, W1, b1, W2, b2, A1, B1, A2, B2,
           trace=False):
    global _last_results
    from concourse.bass_utils import run_bass_kernel_spmd
    if trace:
        _ensure_ntff_hook()

    B, S, _ = hidden_states.shape
    T = B * S
    nc = _get_nc(T)
    in_maps = make_in_maps(hidden_states, Wr, br, W1, b1, W2, b2,
                           A1, B1, A2, B2)
    tmpdir = os.environ.get("KERNEL_TRACE_DIR") or None
    if tmpdir:
        os.makedirs(tmpdir, exist_ok=True)
    res = run_bass_kernel_spmd(nc, in_maps, list(range(NCORES)), trace=trace,
                               tmpdir=tmpdir)
    _last_results = res
    out = np.zeros((T, D), dtype=np.float64)
    for c in range(NCORES):
        out += res.results[c]["outT"].astype(np.float64).T
    return out.astype(np.float32).reshape(B, S, D)
